# revision 6
# baseline (speedup 1.0000x reference)
"""ArcFace loss on 8 TRN2 NeuronCores, sharded along the batch dim B.

Each core owns 64 full rows ([64, 100000] f32 = 25.6 MB), so the
per-row logsumexp and the target-column margin are entirely core-local
— no cross-core collective is needed at all (the baseline's 4 KB
AllGather cost ~23 us of pure end-of-kernel latency). Each core
returns the SUM of its 64 per-row losses; the host unshards by summing
the 8 partial scalars and dividing by B.

Per core: view the shard as [128, 50000] (row r split into two halves
on partitions r and r+64), stream it through ScalarE exp(64*x - 64)
with per-chunk accum_out row-sums, gather each row's target element
via indirect DMA, apply the angular margin on a [64,1] tensor, then
loss_r = 64 + log(sum_r + delta_r) - 64*phi_r where delta corrects the
target column's exp contribution.

Since cosine <= 1, logits are <= 64, so exp(64*c - 64) <= 1 and the
max-pass of logsumexp is unnecessary.
"""

import math
import os

import numpy as np

import concourse.bacc as bacc
import concourse.bass as bass
import concourse.bass_isa as bass_isa
import concourse.mybir as mybir
import concourse.tile as tile
from concourse.bass_utils import run_bass_kernel_spmd

# ArcFace constants (match the reference)
S = 64.0
M = 0.5
COS_M = math.cos(M)
SIN_M = math.sin(M)
TH = math.cos(math.pi - M)
MM = math.sin(math.pi - M) * M
EPS = 1e-07

B, C = 512, 100000
NCORES = 8
BL = B // NCORES  # 64 rows per core
H = 2  # split each row into H halves across partitions
P = BL * H  # 128 partitions
W = C // H  # 50000 columns per partition
FC = int(os.environ.get("K_FC", "3125"))  # stream chunk width
NCH = W // FC
assert NCH * FC == W
SHIFT = 64.0  # exp(S*c - SHIFT) keeps everything <= 1 since c in [-1, 1]

F32 = mybir.dt.float32
I32 = mybir.dt.int32

# DMA ring schedule for the stream: which engine issues chunk i's load.
# "s"=sync (HWDGE ring 0), "a"=scalar/ACT (HWDGE ring 1), "g"=gpsimd (SWDGE).
RING = os.environ.get("K_RING", "sg")


def _patch_act_tables():
    """Make natural_log_exp_and_others the only provider of Exp/Ln so the
    table-load pass emits a single ACT_TABLE_LOAD instead of thrashing
    between the exp-only and ln-only sets."""
    import concourse.hw_specs as hw_specs

    orig = hw_specs.get_activation_tables
    if getattr(orig, "_arcface_patched", False):
        return

    def patched(arch):
        tabs = {k: set(v) for k, v in orig(arch).items()}
        for name, fns in tabs.items():
            if name != "natural_log_exp_and_others":
                fns.discard(mybir.ActivationFunctionType.Exp)
                fns.discard(mybir.ActivationFunctionType.Ln)
        return tabs

    patched._arcface_patched = True
    hw_specs.get_activation_tables = patched
    bacc.get_activation_tables = patched


def build_nc():
    _patch_act_tables()
    nc = bacc.Bacc(None)
    # [2 halves, 64 rows, 50000 cols] — host transposes the [64, 100000]
    # shard so partition p = h*64 + r (halves of row r on partitions r, r+64)
    cos_p = nc.declare_dram_parameter("cosine", [H, BL, W], F32, isOutput=False)
    gidx_p = nc.declare_dram_parameter("gidx", [BL, 1], I32, isOutput=False)
    out_p = nc.declare_dram_parameter("out", [1, 1], F32, isOutput=True)

    cos_flat = cos_p[:].rearrange("h r (c o) -> (h r c) o", o=1)

    ring_engines = {"s": nc.sync, "a": nc.scalar, "g": nc.gpsimd}

    with tile.TileContext(nc) as tc:
        with (
            tc.tile_pool(name="data", bufs=int(os.environ.get("K_BUFS", "3"))) as data_pool,
            tc.tile_pool(name="expp", bufs=2) as exp_pool,
            tc.tile_pool(name="small", bufs=1) as small,
        ):
            # bias operand for exp(S*x - SHIFT) activations
            nbias = small.tile([P, 1], F32)
            nc.gpsimd.memset(nbias[:], -SHIFT)
            # dummy activation: pulls the ACT table load to kernel start
            warm_act = small.tile([P, 1], F32)
            nc.scalar.activation(
                out=warm_act[:], in_=nbias[:], func=mybir.ActivationFunctionType.Exp
            )
            # per-row loss vector; rows BL..127 must stay 0 for the final reduce
            lvec = small.tile([P, 1], F32)
            nc.gpsimd.memset(lvec[:], 0.0)

            # ---- gather target elements (independent of the stream)
            idx_sb = small.tile([P, 1], I32)
            gc = small.tile([P, 1], F32)
            nc.gpsimd.dma_start(out=idx_sb[0:BL, :], in_=gidx_p[:])
            nc.gpsimd.indirect_dma_start(
                out=gc[0:BL, :],
                out_offset=None,
                in_=cos_flat,
                in_offset=bass.IndirectOffsetOnAxis(ap=idx_sb[0:BL, :], axis=0),
            )

            # ---- margin math on [64, 1] tensors (emitted before the stream
            # so the ACT-engine queue finishes these early, off the tail path)
            nb64 = nbias[0:BL, :]
            c = small.tile([P, 1], F32)
            nc.vector.tensor_scalar(
                out=c[0:BL, :], in0=gc[0:BL, :], scalar1=1.0 - EPS, scalar2=-1.0 + EPS,
                op0=mybir.AluOpType.min, op1=mybir.AluOpType.max,
            )
            # om = 1 - c^2
            om = small.tile([P, 1], F32)
            nc.vector.tensor_tensor(
                out=om[0:BL, :], in0=c[0:BL, :], in1=c[0:BL, :], op=mybir.AluOpType.mult
            )
            nc.vector.tensor_scalar(
                out=om[0:BL, :], in0=om[0:BL, :], scalar1=-1.0, scalar2=1.0,
                op0=mybir.AluOpType.mult, op1=mybir.AluOpType.add,
            )
            # sine = exp(0.5 * ln(om)) — avoids the low-precision Sqrt table
            sine = small.tile([P, 1], F32)
            nc.scalar.activation(
                out=sine[0:BL, :], in_=om[0:BL, :], func=mybir.ActivationFunctionType.Ln
            )
            nc.scalar.activation(
                out=sine[0:BL, :], in_=sine[0:BL, :],
                func=mybir.ActivationFunctionType.Exp, scale=0.5,
            )
            # phi = c*COS_M - sine*SIN_M
            phi = small.tile([P, 1], F32)
            t1 = small.tile([P, 1], F32)
            nc.vector.tensor_scalar(
                out=t1[0:BL, :], in0=sine[0:BL, :], scalar1=SIN_M, scalar2=None,
                op0=mybir.AluOpType.mult,
            )
            nc.vector.scalar_tensor_tensor(
                out=phi[0:BL, :], in0=c[0:BL, :], scalar=COS_M, in1=t1[0:BL, :],
                op0=mybir.AluOpType.mult, op1=mybir.AluOpType.subtract,
            )
            # phi = where(c > TH, phi, c - MM)
            gt = small.tile([P, 1], F32)
            nc.vector.tensor_scalar(
                out=gt[0:BL, :], in0=c[0:BL, :], scalar1=TH, scalar2=None,
                op0=mybir.AluOpType.is_gt,
            )
            cmm = small.tile([P, 1], F32)
            nc.vector.tensor_scalar(
                out=cmm[0:BL, :], in0=c[0:BL, :], scalar1=MM, scalar2=None,
                op0=mybir.AluOpType.subtract,
            )
            d = small.tile([P, 1], F32)
            nc.vector.tensor_tensor(
                out=d[0:BL, :], in0=phi[0:BL, :], in1=cmm[0:BL, :],
                op=mybir.AluOpType.subtract,
            )
            nc.vector.tensor_tensor(
                out=d[0:BL, :], in0=d[0:BL, :], in1=gt[0:BL, :], op=mybir.AluOpType.mult
            )
            nc.vector.tensor_tensor(
                out=phi[0:BL, :], in0=cmm[0:BL, :], in1=d[0:BL, :], op=mybir.AluOpType.add
            )
            # tpart = S*phi - SHIFT  (so loss_r = log(s2_r) - tpart_r)
            tpart = small.tile([P, 1], F32)
            nc.vector.tensor_scalar(
                out=tpart[0:BL, :], in0=phi[0:BL, :], scalar1=S, scalar2=-SHIFT,
                op0=mybir.AluOpType.mult, op1=mybir.AluOpType.add,
            )
            # delta = exp(S*phi - SHIFT) - exp(S*c - SHIFT)
            e_phi = small.tile([P, 1], F32)
            e_c = small.tile([P, 1], F32)
            nc.scalar.activation(
                out=e_phi[0:BL, :], in_=phi[0:BL, :],
                func=mybir.ActivationFunctionType.Exp, scale=S, bias=nb64,
            )
            nc.scalar.activation(
                out=e_c[0:BL, :], in_=c[0:BL, :],
                func=mybir.ActivationFunctionType.Exp, scale=S, bias=nb64,
            )
            delta = small.tile([P, 1], F32)
            nc.vector.tensor_tensor(
                out=delta[0:BL, :], in0=e_phi[0:BL, :], in1=e_c[0:BL, :],
                op=mybir.AluOpType.subtract,
            )

            # ---- main streaming pass: exp + per-chunk row-sum accumulate
            sums = small.tile([P, NCH], F32)
            for i in range(NCH):
                off = i * FC
                dt = data_pool.tile([P, FC], F32, tag="data")
                eng = ring_engines[RING[i % len(RING)]]
                eng.dma_start(
                    out=dt[:],
                    in_=cos_p[:, :, off : off + FC].rearrange("h r c -> (h r) c"),
                )
                ev = exp_pool.tile([P, FC], F32, tag="exp")
                nc.scalar.activation(
                    out=ev[:], in_=dt[:],
                    func=mybir.ActivationFunctionType.Exp,
                    scale=S, bias=nbias[:],
                    accum_out=sums[:, i : i + 1],
                )

            # ---- finale: per-row sum across chunks and halves, then loss
            red = small.tile([P, 1], F32)
            nc.vector.tensor_reduce(
                out=red[:], in_=sums[:], axis=mybir.AxisListType.X,
                op=mybir.AluOpType.add,
            )
            # shift half1's row-sums down to partition base 0 (engines can't
            # mix SBUF operands with different base partitions)
            redb = small.tile([P, 1], F32)
            nc.sync.dma_start(out=redb[0:BL, :], in_=red[BL : 2 * BL, :])
            # s2 = (half0 + half1) + delta
            s2 = small.tile([P, 1], F32)
            nc.vector.tensor_tensor(
                out=s2[0:BL, :], in0=red[0:BL, :], in1=redb[0:BL, :],
                op=mybir.AluOpType.add,
            )
            nc.vector.tensor_tensor(
                out=s2[0:BL, :], in0=s2[0:BL, :], in1=delta[0:BL, :],
                op=mybir.AluOpType.add,
            )
            logs = small.tile([P, 1], F32)
            nc.scalar.activation(
                out=logs[0:BL, :], in_=s2[0:BL, :], func=mybir.ActivationFunctionType.Ln
            )
            nc.vector.tensor_tensor(
                out=lvec[0:BL, :], in0=logs[0:BL, :], in1=tpart[0:BL, :],
                op=mybir.AluOpType.subtract,
            )
            ltot = small.tile([P, 1], F32)
            nc.gpsimd.partition_all_reduce(
                ltot[:], lvec[:], channels=P, reduce_op=bass_isa.ReduceOp.add
            )
            nc.sync.dma_start(out=out_p[:], in_=ltot[0:1, :])

    nc.finalize()
    return nc


_CACHE = {}


def _get_nc():
    if "nc" not in _CACHE:
        _CACHE["nc"] = build_nc()
    return _CACHE["nc"]


def make_in_maps(cosine: np.ndarray, labels: np.ndarray):
    labels = np.asarray(labels).astype(np.int64)
    rows = np.arange(BL, dtype=np.int64)
    in_maps = []
    for m in range(NCORES):
        lo = m * BL
        shard = np.ascontiguousarray(
            np.asarray(cosine[lo : lo + BL], dtype=np.float32)
            .reshape(BL, H, W)
            .transpose(1, 0, 2)
        )
        lab = labels[lo : lo + BL]
        # flat index into the [H, BL, W] layout
        gidx = ((lab // W) * (BL * W) + rows * W + (lab % W)).astype(np.int32)
        in_maps.append({"cosine": shard, "gidx": gidx.reshape(BL, 1)})
    return in_maps


def kernel(cosine: np.ndarray, labels: np.ndarray, _trace: bool = False):
    nc = _get_nc()
    in_maps = make_in_maps(np.asarray(cosine, dtype=np.float32), labels)
    res = run_bass_kernel_spmd(
        nc, in_maps, core_ids=list(range(NCORES)), trace=_trace
    )
    # unshard: sum the per-core partial loss sums, divide by batch
    total = 0.0
    for m in range(NCORES):
        total += float(np.asarray(res.results[m]["out"], dtype=np.float32).reshape(()))
    out = np.float32(total / B)
    if _trace:
        return out, res
    return out


# revision 7
# speedup vs baseline: 1.1399x; 1.1399x over previous
"""ArcFace loss on 8 TRN2 NeuronCores, sharded along the batch dim B.

Each core owns 64 full rows ([64, 100000] f32 = 25.6 MB), so the
per-row logsumexp and the target-column margin are entirely core-local
— no cross-core collective is needed at all (a [B,C/8] class-sharded
variant pays ~23 us of end-of-kernel AllGather latency for the
distributed logsumexp). Each core returns the SUM of its 64 per-row
losses; the host unshards by summing the 8 partial scalars and
dividing by B.

Per core: the host lays the shard out as [2, 64, 50000] so partition
p = h*64 + r holds half h of row r. Stream it through ScalarE
exp(64*x - 64) with per-chunk accum_out row-sums, gather each row's
target element via indirect DMA, apply the angular margin on a [64,1]
tensor, then loss_r = 64 + log(sum_r + delta_r) - 64*phi_r where
delta corrects the target column's exp contribution.

Since cosine <= 1, logits are <= 64, so exp(64*c - 64) <= 1 and the
max-pass of logsumexp is unnecessary.
"""

import math
import os

import numpy as np

import concourse.bacc as bacc
import concourse.bass as bass
import concourse.mybir as mybir
import concourse.tile as tile
from concourse.bass_utils import run_bass_kernel_spmd

# ArcFace constants (match the reference)
S = 64.0
M = 0.5
COS_M = math.cos(M)
SIN_M = math.sin(M)
TH = math.cos(math.pi - M)
MM = math.sin(math.pi - M) * M
EPS = 1e-07

B, C = 512, 100000
NCORES = 8
BL = B // NCORES  # 64 rows per core
H = 2  # split each row into H halves across partitions
P = BL * H  # 128 partitions
W = C // H  # 50000 columns per partition
SHIFT = 64.0  # exp(S*c - SHIFT) keeps everything <= 1 since c in [-1, 1]

# stream chunk widths; a short final chunk keeps the last exp off the
# critical tail
_plan_env = os.environ.get("K_PLAN", "6250x7,3125x2")
PLAN = []
for part in _plan_env.split(","):
    wdt, _, rep = part.partition("x")
    PLAN += [int(wdt)] * int(rep or "1")
assert sum(PLAN) == W, PLAN
NCH = len(PLAN)
FCMAX = max(PLAN)

F32 = mybir.dt.float32
I32 = mybir.dt.int32

# DMA ring schedule for the stream: which engine issues chunk i's load.
# "s"=sync (HWDGE ring 0), "a"=scalar/ACT (HWDGE ring 1), "g"=gpsimd (SWDGE).
RING = os.environ.get("K_RING", "sg")
# emit the margin chain after this many stream chunks, so its ACT ops sit
# behind a few stream exps in the in-order ACT queue (by then the indirect
# gather has long completed and they run in ACT idle bubbles, not on the tail)
MARGIN_AFTER = int(os.environ.get("K_MAFTER", "4"))


def _patch_act_tables():
    """Make natural_log_exp_and_others the only provider of Exp/Ln so the
    table-load pass emits a single ACT_TABLE_LOAD instead of thrashing
    between the exp-only and ln-only sets."""
    import concourse.hw_specs as hw_specs

    orig = hw_specs.get_activation_tables
    if getattr(orig, "_arcface_patched", False):
        return

    def patched(arch):
        tabs = {k: set(v) for k, v in orig(arch).items()}
        for name, fns in tabs.items():
            if name != "natural_log_exp_and_others":
                fns.discard(mybir.ActivationFunctionType.Exp)
                fns.discard(mybir.ActivationFunctionType.Ln)
        return tabs

    patched._arcface_patched = True
    hw_specs.get_activation_tables = patched
    bacc.get_activation_tables = patched


def build_nc():
    _patch_act_tables()
    nc = bacc.Bacc(None)
    # [2 halves, 64 rows, 50000 cols] — host transposes the [64, 100000]
    # shard so partition p = h*64 + r (halves of row r on partitions r, r+64)
    cos_p = nc.declare_dram_parameter("cosine", [H, BL, W], F32, isOutput=False)
    gidx_p = nc.declare_dram_parameter("gidx", [BL, 1], I32, isOutput=False)
    out_p = nc.declare_dram_parameter("out", [1, 1], F32, isOutput=True)

    cos_flat = cos_p[:].rearrange("h r (c o) -> (h r c) o", o=1)

    ring_engines = {"s": nc.sync, "a": nc.scalar, "g": nc.gpsimd}

    with tile.TileContext(nc) as tc:
        with (
            tc.tile_pool(name="data", bufs=int(os.environ.get("K_BUFS", "3"))) as data_pool,
            tc.tile_pool(name="expp", bufs=2) as exp_pool,
            tc.tile_pool(name="small", bufs=1) as small,
            tc.tile_pool(name="psum", bufs=1, space="PSUM") as psum,
        ):
            # bias operand for exp(S*x - SHIFT) activations
            nbias = small.tile([P, 1], F32)
            nc.gpsimd.memset(nbias[:], -SHIFT)
            # ones vector for the final TensorE dot-product reduce
            ones = small.tile([P, 1], F32)
            nc.gpsimd.memset(ones[:], 1.0)
            # per-row loss vector; rows BL..127 must stay 0 for the final reduce
            lvec = small.tile([P, 1], F32)
            nc.gpsimd.memset(lvec[:], 0.0)
            # dummy activation: pulls the ACT table load to kernel start
            warm_act = small.tile([P, 1], F32)
            nc.scalar.activation(
                out=warm_act[:], in_=nbias[:], func=mybir.ActivationFunctionType.Exp
            )

            # ---- gather target elements (independent of the stream);
            # idx via the fast HWDGE ring, gather is first in the gpsimd queue
            idx_sb = small.tile([P, 1], I32)
            gc = small.tile([P, 1], F32)
            nc.sync.dma_start(out=idx_sb[0:BL, :], in_=gidx_p[:])
            nc.gpsimd.indirect_dma_start(
                out=gc[0:BL, :],
                out_offset=None,
                in_=cos_flat,
                in_offset=bass.IndirectOffsetOnAxis(ap=idx_sb[0:BL, :], axis=0),
            )

            sums = small.tile([P, NCH], F32)

            def emit_chunk(i, off):
                dt = data_pool.tile([P, FCMAX], F32, tag="data")
                w = PLAN[i]
                eng = ring_engines[RING[i % len(RING)]]
                eng.dma_start(
                    out=dt[:, 0:w],
                    in_=cos_p[:, :, off : off + w].rearrange("h r c -> (h r) c"),
                )
                ev = exp_pool.tile([P, FCMAX], F32, tag="exp")
                nc.scalar.activation(
                    out=ev[:, 0:w], in_=dt[:, 0:w],
                    func=mybir.ActivationFunctionType.Exp,
                    scale=S, bias=nbias[:],
                    accum_out=sums[:, i : i + 1],
                )

            def emit_margin():
                # margin math on [64, 1] tensors; ACT ops here run in the
                # ACT engine's DMA-wait bubbles mid-stream
                nb64 = nbias[0:BL, :]
                c = small.tile([P, 1], F32)
                nc.vector.tensor_scalar(
                    out=c[0:BL, :], in0=gc[0:BL, :], scalar1=1.0 - EPS,
                    scalar2=-1.0 + EPS,
                    op0=mybir.AluOpType.min, op1=mybir.AluOpType.max,
                )
                # om = 1 - c^2
                om = small.tile([P, 1], F32)
                nc.vector.tensor_tensor(
                    out=om[0:BL, :], in0=c[0:BL, :], in1=c[0:BL, :],
                    op=mybir.AluOpType.mult,
                )
                nc.vector.tensor_scalar(
                    out=om[0:BL, :], in0=om[0:BL, :], scalar1=-1.0, scalar2=1.0,
                    op0=mybir.AluOpType.mult, op1=mybir.AluOpType.add,
                )
                # sine = exp(0.5 * ln(om)) — avoids the low-precision Sqrt table
                sine = small.tile([P, 1], F32)
                nc.scalar.activation(
                    out=sine[0:BL, :], in_=om[0:BL, :],
                    func=mybir.ActivationFunctionType.Ln,
                )
                nc.scalar.activation(
                    out=sine[0:BL, :], in_=sine[0:BL, :],
                    func=mybir.ActivationFunctionType.Exp, scale=0.5,
                )
                # phi = c*COS_M - sine*SIN_M
                phi = small.tile([P, 1], F32)
                t1 = small.tile([P, 1], F32)
                nc.vector.tensor_scalar(
                    out=t1[0:BL, :], in0=sine[0:BL, :], scalar1=SIN_M, scalar2=None,
                    op0=mybir.AluOpType.mult,
                )
                nc.vector.scalar_tensor_tensor(
                    out=phi[0:BL, :], in0=c[0:BL, :], scalar=COS_M, in1=t1[0:BL, :],
                    op0=mybir.AluOpType.mult, op1=mybir.AluOpType.subtract,
                )
                # phi = where(c > TH, phi, c - MM)
                gt = small.tile([P, 1], F32)
                nc.vector.tensor_scalar(
                    out=gt[0:BL, :], in0=c[0:BL, :], scalar1=TH, scalar2=None,
                    op0=mybir.AluOpType.is_gt,
                )
                cmm = small.tile([P, 1], F32)
                nc.vector.tensor_scalar(
                    out=cmm[0:BL, :], in0=c[0:BL, :], scalar1=MM, scalar2=None,
                    op0=mybir.AluOpType.subtract,
                )
                d = small.tile([P, 1], F32)
                nc.vector.tensor_tensor(
                    out=d[0:BL, :], in0=phi[0:BL, :], in1=cmm[0:BL, :],
                    op=mybir.AluOpType.subtract,
                )
                nc.vector.tensor_tensor(
                    out=d[0:BL, :], in0=d[0:BL, :], in1=gt[0:BL, :],
                    op=mybir.AluOpType.mult,
                )
                nc.vector.tensor_tensor(
                    out=phi[0:BL, :], in0=cmm[0:BL, :], in1=d[0:BL, :],
                    op=mybir.AluOpType.add,
                )
                # tpart = S*phi - SHIFT  (so loss_r = log(s2_r) - tpart_r)
                tpart = small.tile([P, 1], F32)
                nc.vector.tensor_scalar(
                    out=tpart[0:BL, :], in0=phi[0:BL, :], scalar1=S, scalar2=-SHIFT,
                    op0=mybir.AluOpType.mult, op1=mybir.AluOpType.add,
                )
                # delta = exp(S*phi - SHIFT) - exp(S*c - SHIFT)
                e_phi = small.tile([P, 1], F32)
                e_c = small.tile([P, 1], F32)
                nc.scalar.activation(
                    out=e_phi[0:BL, :], in_=phi[0:BL, :],
                    func=mybir.ActivationFunctionType.Exp, scale=S, bias=nb64,
                )
                nc.scalar.activation(
                    out=e_c[0:BL, :], in_=c[0:BL, :],
                    func=mybir.ActivationFunctionType.Exp, scale=S, bias=nb64,
                )
                delta = small.tile([P, 1], F32)
                nc.vector.tensor_tensor(
                    out=delta[0:BL, :], in0=e_phi[0:BL, :], in1=e_c[0:BL, :],
                    op=mybir.AluOpType.subtract,
                )
                return tpart, delta

            # ---- main streaming pass with the margin chain interleaved
            off = 0
            tpart = delta = None
            for i in range(NCH):
                emit_chunk(i, off)
                off += PLAN[i]
                if i + 1 == MARGIN_AFTER:
                    tpart, delta = emit_margin()
            if tpart is None:
                tpart, delta = emit_margin()

            # ---- finale: per-row sum across chunks and halves, then loss
            red = small.tile([P, 1], F32)
            nc.vector.tensor_reduce(
                out=red[:], in_=sums[:], axis=mybir.AxisListType.X,
                op=mybir.AluOpType.add,
            )
            # shift half1's row-sums down to partition base 0 (engines can't
            # mix SBUF operands with different base partitions)
            redb = small.tile([P, 1], F32)
            nc.sync.dma_start(out=redb[0:BL, :], in_=red[BL : 2 * BL, :])
            # s2 = (half0 + half1) + delta
            s2 = small.tile([P, 1], F32)
            nc.vector.tensor_tensor(
                out=s2[0:BL, :], in0=red[0:BL, :], in1=redb[0:BL, :],
                op=mybir.AluOpType.add,
            )
            nc.vector.tensor_tensor(
                out=s2[0:BL, :], in0=s2[0:BL, :], in1=delta[0:BL, :],
                op=mybir.AluOpType.add,
            )
            logs = small.tile([P, 1], F32)
            nc.scalar.activation(
                out=logs[0:BL, :], in_=s2[0:BL, :],
                func=mybir.ActivationFunctionType.Ln,
            )
            nc.vector.tensor_tensor(
                out=lvec[0:BL, :], in0=logs[0:BL, :], in1=tpart[0:BL, :],
                op=mybir.AluOpType.subtract,
            )
            # partial = ones^T @ lvec on TensorE (gpsimd partition_all_reduce
            # showed a ~7us engine wake-up latency here)
            acc = psum.tile([1, 1], F32)
            nc.tensor.matmul(acc[:], lvec[:], ones[:])
            res = small.tile([1, 1], F32)
            nc.vector.tensor_copy(res[:], acc[:])
            nc.sync.dma_start(out=out_p[:], in_=res[:])

    nc.finalize()
    return nc


_CACHE = {}


def _get_nc():
    if "nc" not in _CACHE:
        _CACHE["nc"] = build_nc()
    return _CACHE["nc"]


def make_in_maps(cosine: np.ndarray, labels: np.ndarray):
    labels = np.asarray(labels).astype(np.int64)
    rows = np.arange(BL, dtype=np.int64)
    in_maps = []
    for m in range(NCORES):
        lo = m * BL
        shard = np.ascontiguousarray(
            np.asarray(cosine[lo : lo + BL], dtype=np.float32)
            .reshape(BL, H, W)
            .transpose(1, 0, 2)
        )
        lab = labels[lo : lo + BL]
        # flat index into the [H, BL, W] layout
        gidx = ((lab // W) * (BL * W) + rows * W + (lab % W)).astype(np.int32)
        in_maps.append({"cosine": shard, "gidx": gidx.reshape(BL, 1)})
    return in_maps


def kernel(cosine: np.ndarray, labels: np.ndarray, _trace: bool = False):
    nc = _get_nc()
    in_maps = make_in_maps(np.asarray(cosine, dtype=np.float32), labels)
    res = run_bass_kernel_spmd(
        nc, in_maps, core_ids=list(range(NCORES)), trace=_trace
    )
    # unshard: sum the per-core partial loss sums, divide by batch
    total = 0.0
    for m in range(NCORES):
        total += float(np.asarray(res.results[m]["out"], dtype=np.float32).reshape(()))
    out = np.float32(total / B)
    if _trace:
        return out, res
    return out


# revision 11
# speedup vs baseline: 1.3528x; 1.1867x over previous
"""ArcFace loss on 8 TRN2 NeuronCores, sharded along the batch dim B.

Each core owns 64 full rows ([64, 100000] f32 = 25.6 MB), so the
per-row logsumexp and the target-column margin are entirely core-local
— no cross-core collective is needed at all (a [B,C/8] class-sharded
variant pays ~23 us of end-of-kernel AllGather latency for the
distributed logsumexp). Each core returns the SUM of its 64 per-row
losses; the host unshards by summing the 8 partial scalars and
dividing by B.

Per core: the host lays the shard out as [2, 64, 50000] so partition
p = h*64 + r holds half h of row r. Stream it through ScalarE
exp(64*x - 64) with per-chunk accum_out row-sums, gather each row's
target element via indirect DMA, apply the angular margin on a [64,1]
tensor, then loss_r = 64 + log(sum_r + delta_r) - 64*phi_r where
delta corrects the target column's exp contribution.

Since cosine <= 1, logits are <= 64, so exp(64*c - 64) <= 1 and the
max-pass of logsumexp is unnecessary.
"""

import math
import os

import numpy as np

import concourse.bacc as bacc
import concourse.bass as bass
import concourse.mybir as mybir
import concourse.tile as tile
from concourse.bass_utils import run_bass_kernel_spmd

# ArcFace constants (match the reference)
S = 64.0
M = 0.5
COS_M = math.cos(M)
SIN_M = math.sin(M)
TH = math.cos(math.pi - M)
MM = math.sin(math.pi - M) * M
EPS = 1e-07

B, C = 512, 100000
NCORES = 8
BL = B // NCORES  # 64 rows per core
H = 2  # split each row into H halves across partitions
P = BL * H  # 128 partitions
W = C // H  # 50000 columns per partition
SHIFT = 64.0  # exp(S*c - SHIFT) keeps everything <= 1 since c in [-1, 1]

# stream chunk widths; short final chunks keep the last exp off the
# critical tail
_plan_env = os.environ.get("K_PLAN", "3125x15,1563,1562")
PLAN = []
for part in _plan_env.split(","):
    wdt, _, rep = part.partition("x")
    PLAN += [int(wdt)] * int(rep or "1")
assert sum(PLAN) == W, PLAN
NCH = len(PLAN)
FCMAX = max(PLAN)

F32 = mybir.dt.float32
I32 = mybir.dt.int32

# DMA ring schedule for the stream: which engine issues chunk i's load.
# "s"=sync (HWDGE ring 0), "a"=scalar/ACT (HWDGE ring 1), "g"=gpsimd (SWDGE).
RING = os.environ.get("K_RING", "sa")
# how many chunks the DMA triggers lead the exp emissions by — keeps
# several transfers queued on the rings so HBM never idles
LEAD = int(os.environ.get("K_LEAD", "3"))
# emit the margin chain after this many stream exps, so its ACT ops sit
# behind a few stream exps in the in-order ACT queue (by then the indirect
# gather has long completed and they run in ACT idle bubbles, not on the tail)
MARGIN_AFTER = int(os.environ.get("K_MAFTER", "3"))


def _patch_act_tables():
    """Make natural_log_exp_and_others the only provider of Exp/Ln so the
    table-load pass emits a single ACT_TABLE_LOAD instead of thrashing
    between the exp-only and ln-only sets."""
    import concourse.hw_specs as hw_specs

    orig = hw_specs.get_activation_tables
    if getattr(orig, "_arcface_patched", False):
        return

    def patched(arch):
        tabs = {k: set(v) for k, v in orig(arch).items()}
        for name, fns in tabs.items():
            if name != "natural_log_exp_and_others":
                fns.discard(mybir.ActivationFunctionType.Exp)
                fns.discard(mybir.ActivationFunctionType.Ln)
        return tabs

    patched._arcface_patched = True
    hw_specs.get_activation_tables = patched
    bacc.get_activation_tables = patched


def build_nc():
    _patch_act_tables()
    nc = bacc.Bacc(None)
    # [2 halves, 64 rows, 50000 cols] — host transposes the [64, 100000]
    # shard so partition p = h*64 + r (halves of row r on partitions r, r+64)
    cos_p = nc.declare_dram_parameter("cosine", [H, BL, W], F32, isOutput=False)
    gidx_p = nc.declare_dram_parameter("gidx", [BL, 1], I32, isOutput=False)
    out_p = nc.declare_dram_parameter("out", [1, 1], F32, isOutput=True)

    cos_flat = cos_p[:].rearrange("h r (c o) -> (h r c) o", o=1)

    ring_engines = {"s": nc.sync, "a": nc.scalar, "g": nc.gpsimd}

    with tile.TileContext(nc) as tc:
        with (
            tc.tile_pool(name="data", bufs=int(os.environ.get("K_BUFS", "6"))) as data_pool,
            tc.tile_pool(name="expp", bufs=2) as exp_pool,
            tc.tile_pool(name="small", bufs=1) as small,
            tc.tile_pool(name="psum", bufs=1, space="PSUM") as psum,
        ):
            # bias operand for exp(S*x - SHIFT) activations
            nbias = small.tile([P, 1], F32)
            nc.gpsimd.memset(nbias[:], -SHIFT)
            # ones vector for the final TensorE dot-product reduce
            ones = small.tile([P, 1], F32)
            nc.gpsimd.memset(ones[:], 1.0)
            # per-row loss vector; rows BL..127 must stay 0 for the final reduce
            lvec = small.tile([P, 1], F32)
            nc.gpsimd.memset(lvec[:], 0.0)
            # dummy activation: pulls the ACT table load to kernel start
            warm_act = small.tile([P, 1], F32)
            nc.scalar.activation(
                out=warm_act[:], in_=nbias[:], func=mybir.ActivationFunctionType.Exp
            )

            # ---- gather target elements (independent of the stream);
            # idx via the fast HWDGE ring, gather is first in the gpsimd queue
            idx_sb = small.tile([P, 1], I32)
            gc = small.tile([P, 1], F32)
            nc.sync.dma_start(out=idx_sb[0:BL, :], in_=gidx_p[:])
            nc.gpsimd.indirect_dma_start(
                out=gc[0:BL, :],
                out_offset=None,
                in_=cos_flat,
                in_offset=bass.IndirectOffsetOnAxis(ap=idx_sb[0:BL, :], axis=0),
            )

            sums = small.tile([P, NCH], F32)
            chunk_tiles = {}

            def emit_dma(i, off):
                dt = data_pool.tile([P, FCMAX], F32, tag="data")
                chunk_tiles[i] = dt
                w = PLAN[i]
                eng = ring_engines[RING[i % len(RING)]]
                eng.dma_start(
                    out=dt[:, 0:w],
                    in_=cos_p[:, :, off : off + w].rearrange("h r c -> (h r) c"),
                )

            def emit_exp(i):
                dt = chunk_tiles.pop(i)
                w = PLAN[i]
                ev = exp_pool.tile([P, FCMAX], F32, tag="exp")
                nc.scalar.activation(
                    out=ev[:, 0:w], in_=dt[:, 0:w],
                    func=mybir.ActivationFunctionType.Exp,
                    scale=S, bias=nbias[:],
                    accum_out=sums[:, i : i + 1],
                )

            def emit_margin():
                # margin math on [64, 1] tensors; ACT ops here run in the
                # ACT engine's DMA-wait bubbles mid-stream
                nb64 = nbias[0:BL, :]
                c = small.tile([P, 1], F32)
                nc.vector.tensor_scalar(
                    out=c[0:BL, :], in0=gc[0:BL, :], scalar1=1.0 - EPS,
                    scalar2=-1.0 + EPS,
                    op0=mybir.AluOpType.min, op1=mybir.AluOpType.max,
                )
                # om = 1 - c^2
                om = small.tile([P, 1], F32)
                nc.vector.tensor_tensor(
                    out=om[0:BL, :], in0=c[0:BL, :], in1=c[0:BL, :],
                    op=mybir.AluOpType.mult,
                )
                nc.vector.tensor_scalar(
                    out=om[0:BL, :], in0=om[0:BL, :], scalar1=-1.0, scalar2=1.0,
                    op0=mybir.AluOpType.mult, op1=mybir.AluOpType.add,
                )
                # sine = exp(0.5 * ln(om)) — avoids the low-precision Sqrt table
                sine = small.tile([P, 1], F32)
                nc.scalar.activation(
                    out=sine[0:BL, :], in_=om[0:BL, :],
                    func=mybir.ActivationFunctionType.Ln,
                )
                nc.scalar.activation(
                    out=sine[0:BL, :], in_=sine[0:BL, :],
                    func=mybir.ActivationFunctionType.Exp, scale=0.5,
                )
                # phi = c*COS_M - sine*SIN_M
                phi = small.tile([P, 1], F32)
                t1 = small.tile([P, 1], F32)
                nc.vector.tensor_scalar(
                    out=t1[0:BL, :], in0=sine[0:BL, :], scalar1=SIN_M, scalar2=None,
                    op0=mybir.AluOpType.mult,
                )
                nc.vector.scalar_tensor_tensor(
                    out=phi[0:BL, :], in0=c[0:BL, :], scalar=COS_M, in1=t1[0:BL, :],
                    op0=mybir.AluOpType.mult, op1=mybir.AluOpType.subtract,
                )
                # phi = where(c > TH, phi, c - MM)
                gt = small.tile([P, 1], F32)
                nc.vector.tensor_scalar(
                    out=gt[0:BL, :], in0=c[0:BL, :], scalar1=TH, scalar2=None,
                    op0=mybir.AluOpType.is_gt,
                )
                cmm = small.tile([P, 1], F32)
                nc.vector.tensor_scalar(
                    out=cmm[0:BL, :], in0=c[0:BL, :], scalar1=MM, scalar2=None,
                    op0=mybir.AluOpType.subtract,
                )
                d = small.tile([P, 1], F32)
                nc.vector.tensor_tensor(
                    out=d[0:BL, :], in0=phi[0:BL, :], in1=cmm[0:BL, :],
                    op=mybir.AluOpType.subtract,
                )
                nc.vector.tensor_tensor(
                    out=d[0:BL, :], in0=d[0:BL, :], in1=gt[0:BL, :],
                    op=mybir.AluOpType.mult,
                )
                nc.vector.tensor_tensor(
                    out=phi[0:BL, :], in0=cmm[0:BL, :], in1=d[0:BL, :],
                    op=mybir.AluOpType.add,
                )
                # tpart = S*phi - SHIFT  (so loss_r = log(s2_r) - tpart_r)
                tpart = small.tile([P, 1], F32)
                nc.vector.tensor_scalar(
                    out=tpart[0:BL, :], in0=phi[0:BL, :], scalar1=S, scalar2=-SHIFT,
                    op0=mybir.AluOpType.mult, op1=mybir.AluOpType.add,
                )
                # delta = exp(S*phi - SHIFT) - exp(S*c - SHIFT)
                e_phi = small.tile([P, 1], F32)
                e_c = small.tile([P, 1], F32)
                nc.scalar.activation(
                    out=e_phi[0:BL, :], in_=phi[0:BL, :],
                    func=mybir.ActivationFunctionType.Exp, scale=S, bias=nb64,
                )
                nc.scalar.activation(
                    out=e_c[0:BL, :], in_=c[0:BL, :],
                    func=mybir.ActivationFunctionType.Exp, scale=S, bias=nb64,
                )
                delta = small.tile([P, 1], F32)
                nc.vector.tensor_tensor(
                    out=delta[0:BL, :], in0=e_phi[0:BL, :], in1=e_c[0:BL, :],
                    op=mybir.AluOpType.subtract,
                )
                return tpart, delta

            # ---- main streaming pass: DMA triggers lead exps by LEAD chunks,
            # margin chain interleaved into an ACT idle bubble
            offs = []
            off = 0
            for wdt in PLAN:
                offs.append(off)
                off += wdt
            tpart = delta = None
            for i in range(NCH):
                emit_dma(i, offs[i])
                if i >= LEAD:
                    emit_exp(i - LEAD)
                    if i - LEAD + 1 == MARGIN_AFTER:
                        tpart, delta = emit_margin()
            for j in range(max(NCH - LEAD, 0), NCH):
                emit_exp(j)
                if j + 1 == MARGIN_AFTER:
                    tpart, delta = emit_margin()
            if tpart is None:
                tpart, delta = emit_margin()

            # ---- finale: per-row sum across chunks and halves, then loss
            red = small.tile([P, 1], F32)
            nc.vector.tensor_reduce(
                out=red[:], in_=sums[:], axis=mybir.AxisListType.X,
                op=mybir.AluOpType.add,
            )
            # shift half1's row-sums down to partition base 0 (engines can't
            # mix SBUF operands with different base partitions)
            redb = small.tile([P, 1], F32)
            nc.sync.dma_start(out=redb[0:BL, :], in_=red[BL : 2 * BL, :])
            # s2 = (half0 + half1) + delta
            s2 = small.tile([P, 1], F32)
            nc.vector.tensor_tensor(
                out=s2[0:BL, :], in0=red[0:BL, :], in1=redb[0:BL, :],
                op=mybir.AluOpType.add,
            )
            nc.vector.tensor_tensor(
                out=s2[0:BL, :], in0=s2[0:BL, :], in1=delta[0:BL, :],
                op=mybir.AluOpType.add,
            )
            logs = small.tile([P, 1], F32)
            nc.scalar.activation(
                out=logs[0:BL, :], in_=s2[0:BL, :],
                func=mybir.ActivationFunctionType.Ln,
            )
            nc.vector.tensor_tensor(
                out=lvec[0:BL, :], in0=logs[0:BL, :], in1=tpart[0:BL, :],
                op=mybir.AluOpType.subtract,
            )
            # partial = ones^T @ lvec on TensorE (gpsimd partition_all_reduce
            # showed a ~7us engine wake-up latency here)
            acc = psum.tile([1, 1], F32)
            nc.tensor.matmul(acc[:], lvec[:], ones[:])
            res = small.tile([1, 1], F32)
            nc.vector.tensor_copy(res[:], acc[:])
            nc.sync.dma_start(out=out_p[:], in_=res[:])

    nc.finalize()
    return nc


_CACHE = {}


def _get_nc():
    if "nc" not in _CACHE:
        _CACHE["nc"] = build_nc()
    return _CACHE["nc"]


def make_in_maps(cosine: np.ndarray, labels: np.ndarray):
    labels = np.asarray(labels).astype(np.int64)
    rows = np.arange(BL, dtype=np.int64)
    in_maps = []
    for m in range(NCORES):
        lo = m * BL
        shard = np.ascontiguousarray(
            np.asarray(cosine[lo : lo + BL], dtype=np.float32)
            .reshape(BL, H, W)
            .transpose(1, 0, 2)
        )
        lab = labels[lo : lo + BL]
        # flat index into the [H, BL, W] layout
        gidx = ((lab // W) * (BL * W) + rows * W + (lab % W)).astype(np.int32)
        in_maps.append({"cosine": shard, "gidx": gidx.reshape(BL, 1)})
    return in_maps


def kernel(cosine: np.ndarray, labels: np.ndarray, _trace: bool = False):
    nc = _get_nc()
    in_maps = make_in_maps(np.asarray(cosine, dtype=np.float32), labels)
    res = run_bass_kernel_spmd(
        nc, in_maps, core_ids=list(range(NCORES)), trace=_trace
    )
    # unshard: sum the per-core partial loss sums, divide by batch
    total = 0.0
    for m in range(NCORES):
        total += float(np.asarray(res.results[m]["out"], dtype=np.float32).reshape(()))
    out = np.float32(total / B)
    if _trace:
        return out, res
    return out
